# revision 2
# baseline (speedup 1.0000x reference)
"""Depthwise cross-correlation (SiamFC-style) Trainium2 kernel.

z: [128, 256, 7, 7] templates, x: [128, 256, 31, 31] search images.
out[b,c,p,q] = sum_{i,j} z[b,c,i,j] * x[b,c,p+i,q+j]  -> [128, 256, 25, 25]

Strategy: pure data parallel over batch (16 batches/core on 8 cores).
Per core: 4096 (b,c) channel pairs = 32 tiles of 128 partitions.
Each tile: 49 diagonal-weight matmuls (one per filter tap) accumulate in
PSUM; rhs is a strided window view of the naturally-laid-out x tile, so
no data replication is needed. float32r streams at 1 col/cycle for N>=256.
"""

import numpy as np

B, C = 128, 256
HZ, WZ = 7, 7
HX, WX = 31, 31
HO, WO = 25, 25
N_CORES = 8
B_PER_CORE = B // N_CORES            # 16
PAIRS = B_PER_CORE * C               # 4096 channel pairs per core
NTILES = PAIRS // 128                # 32
XF = HX * WX                         # 961
ZF = HZ * WZ                         # 49
OF = HO * WO                         # 625
# output p-chunks so each PSUM chunk is one bank (<=512 f32) and N>=256.
# fp32r ISA: innermost free count must be even (moving + psum dst), outer
# steps even -> pad the q-window to 26 (extra col discarded) and pad x rows
# to 32 cols so window reads stay in-tile.
WQ = 26                              # padded q-window (col 25 is garbage)
WXP = 32                             # padded x row pitch
P_SPLIT = 13                         # chunk A: p in [0,13) -> N=338; B: 12*26=312
NA = P_SPLIT * WQ
NB = (HO - P_SPLIT) * WQ
# engine balance: taps 0..ZF-K_OFF-1 run on PE (diag matmul); the last
# K_OFF taps run as ACT products + DVE accumulate. Diag weight builds are
# split DVE/ACT.
K_OFF = 10
N_BUILD_ACT = 12
# emit all chunk-A matmuls, then all chunk-B (avoids PSUM bank ping-pong
# between consecutive MMs); weights stay resident across both passes.
SPLIT_AB = False


def _install_tilefix():
    """This walrus build accepts only one sync-wait command on a Drain.
    Split the TileContext tail-drain waits across single-wait SP nops."""
    import concourse.tile as tile_mod
    from concourse.vector_clock import ScopedClock

    def _drain_and_barrier_split(self, tick_clock, wait_clock):
        nc = self.nc
        probe = nc.sync.nop(nofuse=True, hint="drain_wait_probe")
        wait_clock.add_sem_waits(
            probe.ins, ScopedClock({None: tick_clock.global_clock})
        )
        si = probe.ins.sync_info
        waits = list(si.on_wait) if si is not None and si.on_wait else []
        if si is not None:
            si.on_wait = waits[:1]
        for w in waits[1:]:
            stub = nc.sync.nop(nofuse=True, hint="drain_wait_split")
            ssi = stub.ins.sync_info
            if ssi is None:
                import concourse.mybir as mybir
                stub.ins.sync_info = mybir.SyncInfo(on_wait=[w], on_update=[])
            else:
                ssi.on_wait = list(ssi.on_wait or []) + [w]
        nc.sync.drain()
        nc.all_engine_barrier()
        assert self.sems is not None
        popped = nc._tile_sem_poison_stack.pop()
        assert popped is self._sem_poison
        nc.clear_and_free_semaphores(list(self.sems.allocated().values()))
        nc.all_engine_barrier()

    tile_mod.TileContext._drain_and_barrier = _drain_and_barrier_split


def _split_multi_waits(nc):
    """This walrus build accepts only one sync-wait command per instruction.
    Hoist extra waits onto single-wait nops on the same engine just before."""
    import concourse.mybir as mybir

    n = 0
    for f in nc.m.functions:
        for bb in f.blocks:
            insts = list(bb.instructions)
            out_insts = []
            changed = False
            for inst in insts:
                si = inst.sync_info
                if si is not None and si.on_wait and len(si.on_wait) > 1:
                    waits = list(si.on_wait)
                    si.on_wait = waits[-1:]
                    for w in waits[:-1]:
                        n += 1
                        out_insts.append(mybir.InstNoOp(
                            name=f"waitsplit-{n}",
                            engine=inst.engine,
                            bass_nofuse=True,
                            sync_info=mybir.SyncInfo(on_wait=[w], on_update=[]),
                        ))
                    changed = True
                out_insts.append(inst)
            if changed:
                bb.instructions.clear()
                for inst in out_insts:
                    bb.add_instruction(inst)
    return n


_NC_CACHE = {}


def _build_bass(reps: int = 1, timing: bool = False):
    import concourse.bass as bass
    import concourse.mybir as mybir
    import concourse.tile as tile
    from concourse.masks import make_identity
    from contextlib import ExitStack

    _install_tilefix()

    f32 = mybir.dt.float32
    f32r = mybir.dt.float32r

    nc = bass.Bass()
    xs = nc.declare_dram_parameter("xs", [PAIRS, XF], f32, isOutput=False)
    zs = nc.declare_dram_parameter("zs", [PAIRS, ZF], f32, isOutput=False)
    out_rows = 128 if timing else PAIRS
    out = nc.declare_dram_parameter("out", [out_rows, OF], f32, isOutput=True)

    with tile.TileContext(nc) as tc:
        with (
            tc.tile_pool(name="consts", bufs=1) as consts,
            tc.tile_pool(name="xin", bufs=3) as xin,
            tc.tile_pool(name="zin", bufs=3) as zin,
            tc.tile_pool(name="wts", bufs=(44 if SPLIT_AB else 12)) as wts,
            tc.tile_pool(name="outp", bufs=3) as outp,
            tc.tile_pool(name="accp", bufs=2) as accp,
            tc.tile_pool(name="prodp", bufs=3) as prodp,
            tc.tile_pool(name="psum", bufs=3, space="PSUM") as psum,
        ):
            ident = consts.tile([128, 128], f32)
            make_identity(nc, ident)

            # x is stored row-pitch-32 (even outer stride — the fp32r moving
            # path faults on odd outer strides even though the walrus
            # verifier only checks the innermost dim).
            def win(x_t, i, j, p0, pc, wq):
                return x_t[:, i + p0:i + p0 + pc, j:j + wq]

            for _rep in range(reps):
              for t in range(NTILES):
                r0 = t * 128
                x_t = xin.tile([128, HX, WXP], f32r)
                nc.gpsimd.dma_start(
                    out=x_t[:, :, 0:WX],
                    in_=xs[r0:r0 + 128, :].rearrange("p (h w) -> p h w", h=HX))
                z_t = zin.tile([128, ZF], f32)
                nc.sync.dma_start(out=z_t, in_=zs[r0:r0 + 128, :])

                ps_a = psum.tile([128, P_SPLIT, WQ], f32)
                ps_b = psum.tile([128, HO - P_SPLIT, WQ], f32)

                n_pe = ZF - K_OFF
                w_tiles = []
                for tap in range(n_pe):
                    i, j = divmod(tap, WZ)
                    w = wts.tile([128, 128], f32r)
                    zcol = z_t[:, tap:tap + 1]
                    if tap % 3 == 1 and tap // 3 < N_BUILD_ACT:
                        nc.scalar.mul(w, ident, zcol)
                    else:
                        nc.vector.tensor_scalar_mul(w, ident, zcol)
                    w_tiles.append(w)
                    rhs_a = win(x_t, i, j, 0, P_SPLIT, WQ)
                    nc.tensor.matmul(
                        ps_a, w, rhs_a,
                        start=(tap == 0), stop=(tap == n_pe - 1),
                        skip_group_check=True,
                    )
                    if not SPLIT_AB:
                        rhs_b = win(x_t, i, j, P_SPLIT, HO - P_SPLIT, WQ)
                        nc.tensor.matmul(
                            ps_b, w, rhs_b,
                            start=(tap == 0), stop=(tap == n_pe - 1),
                            skip_group_check=True,
                        )
                if SPLIT_AB:
                    for tap in range(n_pe):
                        i, j = divmod(tap, WZ)
                        rhs_b = win(x_t, i, j, P_SPLIT, HO - P_SPLIT, WQ)
                        nc.tensor.matmul(
                            ps_b, w_tiles[tap], rhs_b,
                            start=(tap == 0), stop=(tap == n_pe - 1),
                            skip_group_check=True,
                        )

                # offloaded taps: ACT per-partition-scaled product, DVE adds
                acc = accp.tile([128, HO, WO], f32)
                x_f = x_t.bitcast(f32)
                for n, tap in enumerate(range(n_pe, ZF)):
                    i, j = divmod(tap, WZ)
                    x_win = win(x_f, i, j, 0, HO, WO)
                    zcol = z_t[:, tap:tap + 1]
                    if n == 0:
                        nc.scalar.mul(acc, x_win, zcol)
                    else:
                        prod = prodp.tile([128, HO, WO], f32)
                        nc.scalar.mul(prod, x_win, zcol)
                        nc.vector.tensor_add(acc, acc, prod)

                o_t = outp.tile([128, HO, WO], f32)
                nc.vector.tensor_add(
                    o_t[:, 0:P_SPLIT, :], acc[:, 0:P_SPLIT, :],
                    ps_a[:, :, 0:WO])
                nc.vector.tensor_add(
                    o_t[:, P_SPLIT:HO, :], acc[:, P_SPLIT:HO, :],
                    ps_b[:, :, 0:WO])
                o0 = 0 if timing else r0
                nc.sync.dma_start(
                    out=out[o0:o0 + 128, :],
                    in_=o_t.rearrange("p h w -> p (h w)"))

    _split_multi_waits(nc)
    return nc


def _get_nc(reps: int = 1, timing: bool = False):
    key = ("nc", reps, timing)
    if key not in _NC_CACHE:
        _NC_CACHE[key] = _build_bass(reps, timing)
    return _NC_CACHE[key]


def _make_in_maps(z: np.ndarray, x: np.ndarray):
    in_maps = []
    for c in range(N_CORES):
        b0 = c * B_PER_CORE
        in_maps.append({
            "xs": x[b0:b0 + B_PER_CORE].reshape(PAIRS, XF),
            "zs": z[b0:b0 + B_PER_CORE].reshape(PAIRS, ZF),
        })
    return in_maps


def kernel(z: np.ndarray, x: np.ndarray, _trace: bool = False):
    from concourse.bass_utils import run_bass_kernel_spmd

    z = np.ascontiguousarray(z, dtype=np.float32)
    x = np.ascontiguousarray(x, dtype=np.float32)
    assert z.shape == (B, C, HZ, WZ) and x.shape == (B, C, HX, WX)

    nc = _get_nc()
    in_maps = _make_in_maps(z, x)
    res = run_bass_kernel_spmd(nc, in_maps, list(range(N_CORES)), trace=_trace)
    out = np.empty((B, C, HO, WO), dtype=np.float32)
    for c in range(N_CORES):
        b0 = c * B_PER_CORE
        out[b0:b0 + B_PER_CORE] = res.results[c]["out"].reshape(
            B_PER_CORE, C, HO, WO)
    if _trace:
        return out, res
    return out



# revision 17
# speedup vs baseline: 1199.8653x; 1199.8653x over previous
"""Depthwise cross-correlation (SiamFC-style) Trainium2 kernel, v2.

z: [128, 256, 7, 7] templates, x: [128, 256, 31, 31] search images.
out[b,c,p,q] = sum_{i,j} z[b,c,i,j] * x[b,c,p+i,q+j]  -> [128, 256, 25, 25]

Pure data parallel over batch (16 batches/core on 8 cores); per core 4096
(b,c) pairs = 32 tiles of 128 partitions.

v2 strategy (vs v1 = 39 fp32r PE diag-matmul taps + ACT-mul/DVE-add
offload, HW-measured 490 us/pass by the same R-slope harness):
  - ALL 49 taps on the PE as bf16 diagonal-weight matmuls (282 us/pass,
    1.74x).  HW measurement showed offload engine ops (ACT mul, DVE
    tensor ops, gpsimd adds) cost ~0.7-1.4 us/tap vs ~0.2 us/tap on PE,
    so every offload lane made the kernel slower; N_DVE/N_ACT/N_GP stay
    available as knobs but default to 0.
  - bf16 halves DMA; PSUM accumulation stays f32; out is written bf16
    (25x26, padded col stripped on host) and upcast host-side.
  - per-tile weight build collapsed into ONE DVE tensor_tensor mult:
      w_all[p, c, t] = ident_rep[p, c, t] * z[p, t]  (z broadcast over c)
    (v1 built each 128x128 diag weight with its own ACT/DVE op)
  - odd-j taps read x_odd (a DMA-made column-shifted copy of x) at j-1,
    keeping every matmul window 4B-aligned.
z is permuted on the host so PE taps occupy columns [0, N_PE) of zs.
"""

import numpy as np

B, C = 128, 256
HZ, WZ = 7, 7
HX, WX = 31, 31
HO, WO = 25, 25
N_CORES = 8
B_PER_CORE = B // N_CORES            # 16
PAIRS = B_PER_CORE * C               # 4096 channel pairs per core
NTILES = PAIRS // 128                # 32
XF = HX * WX                         # 961
ZF = HZ * WZ                         # 49
OF = HO * WO                         # 625
WQ = 26                              # padded q-window (col 25 is garbage)
WOP = 26                             # offload-op width: 52B rows stay 4B-aligned
                                     # so DVE 2x/4x perf modes qualify
WXP = 32                             # padded x row pitch
P_SPLIT = 13                         # PSUM chunk A: p in [0,13); B: [13,25)

# ---- tap assignment ------------------------------------------------------
# Every engine reads 4B-aligned windows: even-j taps window x_t directly;
# odd-j taps window x_odd (x shifted left one column) at j-1.
# (gpsimd has no TensorScalar/STT opcode on this ISA -- its lane is
# TT-adds of ACT-produced products into a private accumulator.)
N_PE = 49                            # diag-matmul taps
N_DVE = 0                            # TS-product + TT-add taps on DVE alone
N_ACT = 0                            # ACT-mul taps, added into o by DVE TT
N_GP = 0                             # ACT-mul taps, added into acc_g by gpsimd
assert N_PE + N_DVE + N_ACT + N_GP == ZF

_all_taps = ([(i, j) for i in range(HZ) for j in range(0, WZ, 2)] +
             [(i, j) for i in range(HZ) for j in range(1, WZ, 2)])


def _tap_lists(n_pe, n_dve, n_act, n_gp):
    assert n_pe + n_dve + n_act + n_gp == ZF
    pe = _all_taps[:n_pe]
    dve = _all_taps[n_pe:n_pe + n_dve]
    act = _all_taps[n_pe + n_dve:n_pe + n_dve + n_act]
    gp = _all_taps[n_pe + n_dve + n_act:]
    return pe, dve, act, gp


PE_TAPS, DVE_TAPS, ACT_TAPS, GP_TAPS = _tap_lists(N_PE, N_DVE, N_ACT, N_GP)
# host-side z column permutation matching the kernel's tap order
Z_PERM = [i * WZ + j for (i, j) in PE_TAPS + DVE_TAPS + ACT_TAPS + GP_TAPS]


def _install_tilefix():
    """This walrus build accepts only one sync-wait command on a Drain.
    Split the TileContext tail-drain waits across single-wait SP nops."""
    import concourse.tile as tile_mod
    from concourse.vector_clock import ScopedClock

    def _drain_and_barrier_split(self, tick_clock, wait_clock):
        nc = self.nc
        probe = nc.sync.nop(nofuse=True, hint="drain_wait_probe")
        wait_clock.add_sem_waits(
            probe.ins, ScopedClock({None: tick_clock.global_clock})
        )
        si = probe.ins.sync_info
        waits = list(si.on_wait) if si is not None and si.on_wait else []
        if si is not None:
            si.on_wait = waits[:1]
        for w in waits[1:]:
            stub = nc.sync.nop(nofuse=True, hint="drain_wait_split")
            ssi = stub.ins.sync_info
            if ssi is None:
                import concourse.mybir as mybir
                stub.ins.sync_info = mybir.SyncInfo(on_wait=[w], on_update=[])
            else:
                ssi.on_wait = list(ssi.on_wait or []) + [w]
        nc.sync.drain()
        nc.all_engine_barrier()
        assert self.sems is not None
        popped = nc._tile_sem_poison_stack.pop()
        assert popped is self._sem_poison
        nc.clear_and_free_semaphores(list(self.sems.allocated().values()))
        nc.all_engine_barrier()

    tile_mod.TileContext._drain_and_barrier = _drain_and_barrier_split


def _split_multi_waits(nc):
    """This walrus build accepts only one sync-wait command per instruction.
    Hoist extra waits onto single-wait nops on the same engine just before."""
    import concourse.mybir as mybir

    n = 0
    for f in nc.m.functions:
        for bb in f.blocks:
            insts = list(bb.instructions)
            out_insts = []
            changed = False
            for inst in insts:
                si = inst.sync_info
                if si is not None and si.on_wait and len(si.on_wait) > 1:
                    waits = list(si.on_wait)
                    si.on_wait = waits[-1:]
                    for w in waits[:-1]:
                        n += 1
                        out_insts.append(mybir.InstNoOp(
                            name=f"waitsplit-{n}",
                            engine=inst.engine,
                            bass_nofuse=True,
                            sync_info=mybir.SyncInfo(on_wait=[w], on_update=[]),
                        ))
                    changed = True
                out_insts.append(inst)
            if changed:
                bb.instructions.clear()
                for inst in out_insts:
                    bb.add_instruction(inst)
    return n


_NC_CACHE = {}


def _build_bass(reps: int = 1, timing: bool = False):
    import os
    import concourse.bass as bass
    import concourse.mybir as mybir
    import concourse.tile as tile
    from concourse.masks import make_identity

    _install_tilefix()

    f32 = mybir.dt.float32
    bf16 = mybir.dt.bfloat16
    MUL = mybir.AluOpType.mult
    ADD = mybir.AluOpType.add

    # experiment knob (timing experiments only; default = module constants)
    split = os.environ.get("K_SPLIT")
    if split:
        pe_taps, dve_taps, act_taps, gp_taps = _tap_lists(
            *(int(v) for v in split.split(",")))
    else:
        pe_taps, dve_taps, act_taps, gp_taps = (
            PE_TAPS, DVE_TAPS, ACT_TAPS, GP_TAPS)
    n_pe = len(pe_taps)
    n_offload = len(dve_taps) + len(act_taps) + len(gp_taps)
    nc = bass.Bass()
    xs = nc.declare_dram_parameter("xs", [PAIRS, XF], bf16, isOutput=False)
    zs = nc.declare_dram_parameter("zs", [PAIRS, ZF], f32, isOutput=False)
    out_rows = 128 if timing else PAIRS
    out = nc.declare_dram_parameter("out", [out_rows, HO * WOP], bf16, isOutput=True)

    with tile.TileContext(nc) as tc:
        with (
            tc.tile_pool(name="consts", bufs=1) as consts,
            tc.tile_pool(name="xin", bufs=3) as xin,
            tc.tile_pool(name="zin", bufs=3) as zin,
            tc.tile_pool(name="zbf", bufs=3) as zbf,
            tc.tile_pool(name="wts", bufs=2) as wts,
            tc.tile_pool(name="tmpp", bufs=3) as tmpp,
            tc.tile_pool(name="outp", bufs=3) as outp,
            tc.tile_pool(name="accg", bufs=3) as accg,
            tc.tile_pool(name="prodp", bufs=12) as prodp,
            tc.tile_pool(name="xodd", bufs=3) as xodd,
            tc.tile_pool(name="psum", bufs=4, space="PSUM") as psum,
        ):
            ident = consts.tile([128, 128], bf16)
            make_identity(nc, ident)
            # ident replicated along an inner tap axis: ident_rep[p, c, t]
            # = (p == c).  One-time cost; lets the per-tile weight build be
            # a single packed-innermost TT multiply.
            WB = n_pe + (n_pe % 2)       # even width: aligned packed rows
            ident_rep = consts.tile([128, 128, WB], bf16)
            for k in range(WB):
                nc.vector.tensor_copy(ident_rep[:, :, k], ident)

            def win(x_t, x_o, i, j, p0, pc, wq):
                # 4B-aligned window: odd j reads the shifted copy at j-1
                if j % 2 == 0:
                    return x_t[:, i + p0:i + p0 + pc, j:j + wq]
                return x_o[:, i + p0:i + p0 + pc, j - 1:j - 1 + wq]

            for _rep in range(reps):
              for t in range(NTILES):
                r0 = t * 128
                x_t = xin.tile([128, HX, WXP], bf16)
                nc.sync.dma_start(
                    out=x_t[:, :, 0:WX],
                    in_=xs[r0:r0 + 128, :].rearrange("p (h w) -> p h w", h=HX))
                # column-shifted copy: x_odd[:, r, q] = x[:, r, q+1]
                x_o = xodd.tile([128, HX, WXP], bf16)
                nc.sync.dma_start(out=x_o[:, :, 0:WX - 1],
                                  in_=x_t[:, :, 1:WX])
                z_t = zin.tile([128, ZF + 1], f32)
                nc.sync.dma_start(out=z_t[:, 0:ZF], in_=zs[r0:r0 + 128, :])

                # ---- one-op diag weight build for all PE taps ----
                z_bf = zbf.tile([128, WB], bf16)
                nc.scalar.copy(z_bf, z_t[:, 0:WB])
                w_all = wts.tile([128, 128, WB], bf16)
                z_pe = z_bf.unsqueeze(1).to_broadcast(
                    (128, 128, WB))
                nc.vector.tensor_tensor(w_all, ident_rep, z_pe, MUL)

                ps_a = psum.tile([128, P_SPLIT, WQ], f32)
                ps_b = psum.tile([128, HO - P_SPLIT, WQ], f32)

                # ---- offload products first (ACT muls feed DVE+GP) ----
                o_t = outp.tile([128, HO, WOP], bf16)
                zi = n_pe
                for n, (i, j) in enumerate(dve_taps):
                    x_win = win(x_t, x_o, i, j, 0, HO, WOP)
                    zcol = z_t[:, zi + n:zi + n + 1]
                    if n == 0:
                        nc.vector.tensor_scalar_mul(o_t, x_win, zcol)
                    else:
                        prod = prodp.tile([128, HO, WOP], bf16)
                        nc.vector.tensor_scalar_mul(prod, x_win, zcol)
                        nc.vector.tensor_tensor(o_t, o_t, prod, ADD)

                zi += len(dve_taps)
                for n, (i, j) in enumerate(act_taps):
                    x_win = win(x_t, x_o, i, j, 0, HO, WOP)
                    zcol = z_t[:, zi + n:zi + n + 1]
                    prod = prodp.tile([128, HO, WOP], bf16)
                    nc.scalar.mul(prod, x_win, zcol)
                    if n == 0 and not dve_taps:
                        nc.vector.tensor_copy(o_t, prod)
                    else:
                        nc.vector.tensor_tensor(o_t, o_t, prod, ADD)

                zi += len(act_taps)
                if gp_taps:
                    acc_g = accg.tile([128, HO, WOP], bf16)
                    gprods = []
                    for n, (i, j) in enumerate(gp_taps):
                        x_win = win(x_t, x_o, i, j, 0, HO, WOP)
                        zcol = z_t[:, zi + n:zi + n + 1]
                        prod = prodp.tile([128, HO, WOP], bf16)
                        nc.scalar.mul(prod, x_win, zcol)
                        gprods.append(prod)
                    if len(gprods) >= 2:
                        nc.gpsimd.tensor_tensor(acc_g, gprods[0], gprods[1],
                                                ADD)
                        for prod in gprods[2:]:
                            nc.gpsimd.tensor_tensor(acc_g, acc_g, prod, ADD)
                    else:
                        acc_g = gprods[0]

                # ---- PE diag-matmul taps ----
                for k, (i, j) in enumerate(pe_taps):
                    w = w_all[:, :, k]
                    rhs_a = win(x_t, x_o, i, j, 0, P_SPLIT, WQ)
                    nc.tensor.matmul(
                        ps_a, w, rhs_a,
                        start=(k == 0), stop=(k == n_pe - 1),
                        skip_group_check=True,
                    )
                    rhs_b = win(x_t, x_o, i, j, P_SPLIT, HO - P_SPLIT, WQ)
                    nc.tensor.matmul(
                        ps_b, w, rhs_b,
                        start=(k == 0), stop=(k == n_pe - 1),
                        skip_group_check=True,
                    )

                # ---- ACT: PSUM f32 -> SBUF bf16 (so combines run fast) ----
                tmp = tmpp.tile([128, HO, WOP], bf16)
                nc.scalar.copy(tmp[:, 0:P_SPLIT, :], ps_a)
                nc.scalar.copy(tmp[:, P_SPLIT:HO, :], ps_b)

                # ---- combines (DVE, all-SBUF bf16) ----
                if n_offload:
                    nc.vector.tensor_tensor(o_t, o_t, tmp, ADD)
                    if gp_taps:
                        nc.vector.tensor_tensor(o_t, o_t, acc_g, ADD)
                else:
                    o_t = tmp

                o0 = 0 if timing else r0
                nc.sync.dma_start(
                    out=out[o0:o0 + 128, :],
                    in_=o_t.rearrange("p h w -> p (h w)"))

    _split_multi_waits(nc)
    return nc


def _get_nc(reps: int = 1, timing: bool = False):
    key = ("nc", reps, timing)
    if key not in _NC_CACHE:
        _NC_CACHE[key] = _build_bass(reps, timing)
    return _NC_CACHE[key]


def _make_in_maps(z: np.ndarray, x: np.ndarray):
    import ml_dtypes

    bf = ml_dtypes.bfloat16
    zp = np.ascontiguousarray(z.reshape(B, C, ZF)[:, :, Z_PERM],
                              dtype=np.float32)
    xb = x.astype(bf)
    in_maps = []
    for c in range(N_CORES):
        b0 = c * B_PER_CORE
        in_maps.append({
            "xs": xb[b0:b0 + B_PER_CORE].reshape(PAIRS, XF),
            "zs": zp[b0:b0 + B_PER_CORE].reshape(PAIRS, ZF),
        })
    return in_maps


def kernel(z: np.ndarray, x: np.ndarray, _trace: bool = False):
    from concourse.bass_utils import run_bass_kernel_spmd

    z = np.ascontiguousarray(z, dtype=np.float32)
    x = np.ascontiguousarray(x, dtype=np.float32)
    assert z.shape == (B, C, HZ, WZ) and x.shape == (B, C, HX, WX)

    nc = _get_nc()
    in_maps = _make_in_maps(z, x)
    res = run_bass_kernel_spmd(nc, in_maps, list(range(N_CORES)), trace=_trace)
    out = np.empty((B, C, HO, WO), dtype=np.float32)
    for c in range(N_CORES):
        b0 = c * B_PER_CORE
        out[b0:b0 + B_PER_CORE] = np.asarray(
            res.results[c]["out"], dtype=np.float32).reshape(
            B_PER_CORE, C, HO, WOP)[:, :, :, 0:WO]
    if _trace:
        return out, res
    return out
